# revision 1
# baseline (speedup 1.0000x reference)
"""Multi-head attention (B=2, S=2048, D=1024, H=16, causal + rel-pos-bias + RoPE)
on 8 Trainium2 NeuronCores.

Sharding: core c handles batch c//4 and head-group c%4 (4 heads = 256 model dims).
Each core computes its heads' Q/K/V projections (column-sharded weights), RoPE,
causal attention with relative position bias, and a partial output projection
(row-sharded Wo). Host sums the 4 partials per batch and adds Wo_b.
"""

import math

import numpy as np
import ml_dtypes

import concourse.bass as bass
import concourse.mybir as mybir
import concourse.tile as tile
from concourse import bacc
from concourse.bass_utils import run_bass_kernel_spmd

BF16 = ml_dtypes.bfloat16

B, S, D, H = 2, 2048, 1024, 16
DK = 64
SCALE = math.sqrt(DK)
HPC = 4          # heads per core
GDIM = HPC * DK  # 256 model dims per core
N_CORES = 8
KT = S // 128    # 16 k-tiles
QC = S // 512    # 4 q-chunks

f32 = mybir.dt.float32
f32r = mybir.dt.float32r
bf16 = mybir.dt.bfloat16


def _sched():
    """Attention tile schedule, shared by host bias packer and device builder.

    Yields (h, qc, kt, n, q0): head-local index, q-chunk, k-tile, the valid
    column count and starting q of the S^T tile [128 k, n q]."""
    for h in range(HPC):
        for qc in range(QC):
            for kt in range(4 * qc + 4):
                if kt // 4 == qc:  # diagonal-crossing tile
                    n = 512 - 128 * (kt % 4)
                    q0 = 128 * kt
                else:
                    n = 512
                    q0 = 512 * qc
                yield h, qc, kt, n, q0


EB_PER_HEAD = sum(128 * n for h, qc, kt, n, q0 in _sched()) // HPC
EB_TOTAL = EB_PER_HEAD * HPC

_PROGRAM = None


def _quads(qc):
    """kt quad-groups for one (h, qc) chunk: list of (kt_list, [(kt,n,q0)...])."""
    kts = list(range(4 * qc + 4))
    out = []
    for i in range(0, len(kts), 4):
        grp = []
        for kt in kts[i:i + 4]:
            if kt // 4 == qc:
                n = 512 - 128 * (kt % 4)
                q0 = 128 * kt
            else:
                n = 512
                q0 = 512 * qc
            grp.append((kt, n, q0))
        out.append(grp)
    return out


def _build_program():
    nc = bacc.Bacc("TRN2", target_bir_lowering=False, debug=False)

    dqT = nc.dram_tensor("qT", [8, 128, S], bf16, kind="ExternalInput").ap()
    dkT = nc.dram_tensor("kT", [8, 128, S], bf16, kind="ExternalInput").ap()
    dvT = nc.dram_tensor("vT", [8, 128, S], bf16, kind="ExternalInput").ap()
    dwq = nc.dram_tensor("wq", [8, 128, GDIM], bf16, kind="ExternalInput").ap()
    dwk = nc.dram_tensor("wk", [8, 128, GDIM], bf16, kind="ExternalInput").ap()
    dwv = nc.dram_tensor("wv", [8, 128, GDIM], bf16, kind="ExternalInput").ap()
    dwo = nc.dram_tensor("wo", [2, 128, D], bf16, kind="ExternalInput").ap()
    deb = nc.dram_tensor("eb", [EB_TOTAL], bf16, kind="ExternalInput").ap()
    dcos = nc.dram_tensor("cosT", [128, S], bf16, kind="ExternalInput").ap()
    dsin = nc.dram_tensor("sinT", [128, S], bf16, kind="ExternalInput").ap()
    dout = nc.dram_tensor("out", [S, D], f32, kind="ExternalOutput").ap()

    with tile.TileContext(nc) as tc:
        with tc.tile_pool(name="consts", bufs=1) as consts, \
             tc.tile_pool(name="persist", bufs=1) as persist, \
             tc.tile_pool(name="ropep", bufs=2) as ropep, \
             tc.tile_pool(name="attn_sb", bufs=2) as attn_sb, \
             tc.tile_pool(name="normp", bufs=2) as normp, \
             tc.tile_pool(name="outst", bufs=3) as outst, \
             tc.tile_pool(name="xf", bufs=1) as xf, \
             tc.tile_pool(name="psum", bufs=1, space="PSUM") as psum:

            # ---- constants & resident activations ----
            wq_s = consts.tile([128, 8, GDIM], bf16)
            wk_s = consts.tile([128, 8, GDIM], bf16)
            wv_s = consts.tile([128, 8, GDIM], bf16)
            wo_s = consts.tile([128, 2, D], bf16)
            cos_s = consts.tile([128, S], bf16)
            sin_s = consts.tile([128, S], bf16)
            vfull = consts.tile([128, 8, S], bf16)
            nc.scalar.dma_start(out=cos_s, in_=dcos)
            nc.scalar.dma_start(out=sin_s, in_=dsin)
            for t in range(8):
                nc.scalar.dma_start(out=wv_s[:, t, :], in_=dwv[t])
                nc.scalar.dma_start(out=vfull[:, t, :], in_=dvT[t])
            for t in range(8):
                nc.gpsimd.dma_start(out=wk_s[:, t, :], in_=dwk[t])
            for t in range(2):
                nc.gpsimd.dma_start(out=wo_s[:, t, :], in_=dwo[t])

            ones_f = consts.tile([1, DK], f32)
            nc.vector.memset(ones_f, 1.0)
            ones_r = consts.tile([1, DK], f32r)
            nc.vector.tensor_copy(out=ones_r, in_=ones_f)

            QT = [persist.tile([128, S], bf16, name=f"QT{m}") for m in range(2)]
            KTt = [persist.tile([128, S], bf16, name=f"KTt{m}") for m in range(2)]
            Vt = persist.tile([128, KT, HPC, DK + 1], bf16)
            cxT = [persist.tile([128, S], bf16, name=f"cxT{m}") for m in range(2)]
            nc.vector.memset(Vt[:, :, :, DK:DK + 1], 1.0)

            def rope(pp, dst, n):
                # pp: psum [128,512] raw proj.T for q-chunk n; dst sbuf [128,S]
                sw = ropep.tile([128, 512], bf16, tag="sw")
                for base in (0, 64):
                    nc.scalar.copy(out=sw[base:base + 32, :],
                                   in_=pp[base + 32:base + 64, :])
                    nc.scalar.copy(out=sw[base + 32:base + 64, :],
                                   in_=pp[base:base + 32, :])
                ss = ropep.tile([128, 512], bf16, tag="ss")
                nc.vector.tensor_mul(out=ss, in0=sw,
                                     in1=sin_s[:, 512 * n:512 * n + 512])
                cc = ropep.tile([128, 512], bf16, tag="cc")
                nc.vector.tensor_mul(out=cc, in0=pp,
                                     in1=cos_s[:, 512 * n:512 * n + 512])
                nc.vector.tensor_add(out=dst[:, 512 * n:512 * n + 512],
                                     in0=cc, in1=ss)

            # ---- projections: Q and K in 2 waves of 4 psum banks ----
            xq = xf.tile([128, 8, S], bf16, tag="xfull", name="xq")
            for t in range(8):
                nc.sync.dma_start(out=wq_s[:, t, :], in_=dwq[t])
                nc.sync.dma_start(out=xq[:, t, :], in_=dqT[t])
            xk = xf.tile([128, 8, S], bf16, tag="xfull", name="xk")
            for t in range(8):
                nc.gpsimd.dma_start(out=xk[:, t, :], in_=dkT[t])
            for which, wsrc, xsrc, dsts in (("q", wq_s, xq, QT),
                                            ("k", wk_s, xk, KTt)):
                for m in range(2):
                    for w in range(2):
                        pp = [psum.tile([128, 512], f32, tag="pctx", bufs=4,
                                        name=f"pp{which}{m}{w}{n}")
                              for n in (2 * w, 2 * w + 1)]
                        for t in range(8):
                            for j, n in enumerate((2 * w, 2 * w + 1)):
                                nc.tensor.matmul(
                                    pp[j],
                                    lhsT=wsrc[:, t, 128 * m:128 * m + 128],
                                    rhs=xsrc[:, t, 512 * n:512 * n + 512],
                                    start=(t == 0), stop=(t == 7))
                        for j, n in enumerate((2 * w, 2 * w + 1)):
                            rope(pp[j], dsts[m], n)
                if which == "q":
                    # V projection overlaps Q-rope tail
                    for tt in range(KT):
                        pv = psum.tile([128, GDIM], f32, tag="pctx", bufs=4,
                                       name="pv")
                        for t in range(8):
                            nc.tensor.matmul(
                                pv,
                                lhsT=vfull[:, t, 128 * tt:128 * tt + 128],
                                rhs=wv_s[:, t, :],
                                start=(t == 0), stop=(t == 7))
                        nc.vector.tensor_copy(
                            out=Vt[:, tt, :, 0:DK],
                            in_=pv.rearrange("p (h d) -> p h d", h=HPC))

            # ---- attention (qc-major) fused with output projection ----
            # per-head packed-bias offset of each qc block
            woff_qc = []
            acc = 0
            for qc in range(QC):
                woff_qc.append(acc)
                for grp in _quads(qc):
                    acc += 128 * sum(n for kt, n, q0 in grp)
            assert acc == EB_PER_HEAD

            for qc in range(QC):
                for m in range(2):          # head pair (2m, 2m+1)
                    pcx = [psum.tile([DK + 1, 512], f32, tag="pctx", bufs=4,
                                     name=f"pcx{a}") for a in range(2)]
                    last_kt = 4 * qc + 3
                    woff = woff_qc[qc]
                    for grp in _quads(qc):
                        gn = sum(n for kt, n, q0 in grp)
                        praw = [attn_sb.tile([128, gn], bf16, tag=f"praw{a}", bufs=3,
                                             name=f"praw{a}") for a in range(2)]
                        ebt = [attn_sb.tile([128, gn], bf16, tag=f"ebt{a}", bufs=3,
                                            name=f"ebt{a}") for a in range(2)]
                        for a in range(2):
                            base = (2 * m + a) * EB_PER_HEAD + woff
                            nc.sync.dma_start(
                                out=ebt[a],
                                in_=deb[base:base + 128 * gn].rearrange(
                                    "(p n) -> p n", p=128))
                        woff += 128 * gn
                        goff = 0
                        for pi in range(0, len(grp), 2):
                            pair = grp[pi:pi + 2]
                            pn = sum(n for kt, n, q0 in pair)
                            pss = [psum.tile([128, pn], f32, tag="ps", bufs=2,
                                             name=f"ps{a}") for a in range(2)]
                            for a in range(2):
                                soff = 0
                                for kt, n, q0 in pair:
                                    nc.tensor.matmul(
                                        pss[a][:, soff:soff + n],
                                        lhsT=KTt[m][64 * a:64 * a + DK,
                                                    128 * kt:128 * kt + 128],
                                        rhs=QT[m][64 * a:64 * a + DK,
                                                  q0:q0 + n],
                                        start=True, stop=True,
                                        tile_position=(64 * a, 0))
                                    soff += n
                            for a in range(2):
                                nc.scalar.activation(
                                    out=praw[a][:, goff:goff + pn], in_=pss[a],
                                    func=mybir.ActivationFunctionType.Exp)
                            goff += pn
                        for a in range(2):
                            nc.vector.tensor_mul(out=praw[a], in0=praw[a],
                                                 in1=ebt[a])
                        goff = 0
                        for kt, n, q0 in grp:
                            co = q0 - 512 * qc
                            for a in range(2):
                                nc.tensor.matmul(
                                    pcx[a][:, co:co + n],
                                    lhsT=Vt[:, kt, 2 * m + a, :],
                                    rhs=praw[a][:, goff:goff + n],
                                    start=(kt == 0), stop=(kt == last_kt))
                            goff += n
                    # normalize per head: ctx_a /= l_a
                    for a in range(2):
                        lrow = normp.tile([1, 512], f32, tag="lrow", bufs=2,
                                          name="lrow")
                        nc.vector.tensor_copy(out=lrow,
                                              in_=pcx[a][DK:DK + 1, :])
                        rec_f = normp.tile([1, 512], f32, tag="rec_f", bufs=2,
                                           name="rec_f")
                        nc.vector.reciprocal_approx_fast(out=rec_f, in_=lrow)
                        rec = normp.tile([1, 512], f32r, tag="rec", bufs=2,
                                         name="rec")
                        nc.vector.tensor_copy(out=rec, in_=rec_f)
                        pb = psum.tile([DK, 512], f32, tag="pctx", bufs=4,
                                       name="pb")
                        nc.tensor.matmul(pb, lhsT=ones_r, rhs=rec,
                                         start=True, stop=True)
                        bc = normp.tile([DK, 512], f32, tag="bc", name="bc", bufs=3)
                        nc.vector.tensor_copy(out=bc, in_=pb)
                        nc.vector.tensor_mul(
                            out=cxT[m][64 * a:64 * a + DK,
                                       512 * qc:512 * qc + 512],
                            in0=pcx[a][0:DK, :], in1=bc)

                # output projection for this qc's 4 token tiles
                for tt in range(4 * qc, 4 * qc + 4):
                    po = [psum.tile([128, 512], f32, tag="pctx", bufs=4,
                                    name=f"po{e}") for e in range(2)]
                    for m in range(2):
                        for e in range(2):
                            nc.tensor.matmul(
                                po[e],
                                lhsT=cxT[m][:, 128 * tt:128 * tt + 128],
                                rhs=wo_s[:, m, 512 * e:512 * e + 512],
                                start=(m == 0), stop=(m == 1))
                    ost = outst.tile([128, D], f32, tag="ost")
                    nc.vector.tensor_copy(out=ost[:, 0:512], in_=po[0])
                    nc.vector.tensor_copy(out=ost[:, 512:1024], in_=po[1])
                    nc.sync.dma_start(out=dout[128 * tt:128 * tt + 128, :],
                                      in_=ost)

    nc.compile()
    return nc


def _get_program():
    global _PROGRAM
    if _PROGRAM is None:
        _PROGRAM = _build_program()
    return _PROGRAM


def _rope_tables():
    half = DK // 2
    inv_freq = 1.0 / (10000.0 ** (np.arange(half, dtype=np.float64) / half))
    ang = np.arange(S, dtype=np.float64)[:, None] * inv_freq[None, :]  # [S, 32]
    cos = np.cos(ang).T  # [32, S]
    sin = np.sin(ang).T
    cos64 = np.concatenate([cos, cos], axis=0)            # [64, S]
    sin64 = np.concatenate([-sin, sin], axis=0)           # signed for rotate-half
    cosT = np.tile(cos64, (2, 1)).astype(BF16)            # [128, S]
    sinT = np.tile(sin64, (2, 1)).astype(BF16)
    return np.ascontiguousarray(cosT), np.ascontiguousarray(sinT)


def _pack_ebias(bias_g):
    """bias_g: [HPC, S, S] f32 (this group's heads). Returns packed 1D bf16,
    one contiguous [128, gn] row-major block per kt-quad (matching the wide
    SBUF tiles the kernel DMAs)."""
    out = np.empty(EB_TOTAL, dtype=BF16)
    off = 0
    tri = np.triu(np.ones((128, 128), dtype=np.float32))
    for h in range(HPC):
        for qc in range(QC):
            for grp in _quads(qc):
                blks = []
                for kt, n, q0 in grp:
                    blk = np.exp(
                        bias_g[h, q0:q0 + n, 128 * kt:128 * kt + 128]
                        .astype(np.float64)).T.astype(np.float32)  # [128, n]
                    if kt // 4 == qc:
                        blk[:, 0:128] *= tri
                    blks.append(blk)
                wide = np.concatenate(blks, axis=1)  # [128, gn]
                gn = wide.shape[1]
                out[off:off + 128 * gn] = wide.astype(BF16).reshape(-1)
                off += 128 * gn
    assert off == EB_TOTAL
    return out


def _prep_inputs(query, key, value, rel_pos_bias, Wq, Wk, Wv, Wo_w):
    cosT, sinT = _rope_tables()
    xT = {}
    for nm, x in (("q", query), ("k", key), ("v", value)):
        for b in range(B):
            t = np.ascontiguousarray(x[b].T.reshape(8, 128, S)).astype(BF16)
            xT[(nm, b)] = t
    wqs, wks, wvs, wos, ebs = {}, {}, {}, {}, {}
    for g in range(4):
        sl = slice(GDIM * g, GDIM * (g + 1))
        wqs[g] = np.ascontiguousarray(
            (Wq[sl, :] / SCALE).T.reshape(8, 128, GDIM)).astype(BF16)
        wks[g] = np.ascontiguousarray(Wk[sl, :].T.reshape(8, 128, GDIM)).astype(BF16)
        wvs[g] = np.ascontiguousarray(Wv[sl, :].T.reshape(8, 128, GDIM)).astype(BF16)
        wos[g] = np.ascontiguousarray(Wo_w[:, sl].T.reshape(2, 128, D)).astype(BF16)
        ebs[g] = _pack_ebias(rel_pos_bias[0, HPC * g:HPC * (g + 1)])
    in_maps = []
    for c in range(N_CORES):
        b, g = c // 4, c % 4
        in_maps.append({
            "qT": xT[("q", b)], "kT": xT[("k", b)], "vT": xT[("v", b)],
            "wq": wqs[g], "wk": wks[g], "wv": wvs[g], "wo": wos[g],
            "eb": ebs[g], "cosT": cosT, "sinT": sinT,
        })
    return in_maps


def _run(query, key, value, rel_pos_bias, Wq, Wk, Wv, Wo_w, Wo_b, trace=False,
         **trace_kwargs):
    nc = _get_program()
    in_maps = _prep_inputs(query, key, value, rel_pos_bias, Wq, Wk, Wv, Wo_w)
    res = run_bass_kernel_spmd(nc, in_maps, core_ids=list(range(N_CORES)),
                               trace=trace, **trace_kwargs)
    out = np.empty((B, S, D), dtype=np.float32)
    for b in range(B):
        acc = res.results[4 * b]["out"].astype(np.float32)
        for g in range(1, 4):
            acc = acc + res.results[4 * b + g]["out"]
        out[b] = acc + Wo_b[None, :]
    return out, res


def _cpu_fallback(query, key, value, mask, rel_pos_bias, Wq, Wk, Wv, Wo_w, Wo_b):
    def rope_np(x):
        half = DK // 2
        inv_freq = 1.0 / (10000.0 ** (np.arange(half, dtype=np.float32) / half))
        ang = np.arange(S, dtype=np.float32)[:, None] * inv_freq[None, :]
        cos = np.concatenate([np.cos(ang), np.cos(ang)], axis=-1)[None, None]
        sin = np.concatenate([np.sin(ang), np.sin(ang)], axis=-1)[None, None]
        x1, x2 = x[..., :half], x[..., half:]
        rot = np.concatenate([-x2, x1], axis=-1)
        return x * cos + rot * sin

    q = np.einsum('bsd,ed->bse', query, Wq).reshape(B, S, H, DK).transpose(0, 2, 1, 3)
    k = np.einsum('bsd,ed->bse', key, Wk).reshape(B, S, H, DK).transpose(0, 2, 1, 3)
    v = np.einsum('bsd,ed->bse', value, Wv).reshape(B, S, H, DK).transpose(0, 2, 1, 3)
    q, k = rope_np(q), rope_np(k)
    sc = np.einsum('bhqd,bhkd->bhqk', q, k) / SCALE + rel_pos_bias
    sc = np.where(mask, sc, -np.inf)
    sc = sc - sc.max(axis=-1, keepdims=True)
    e = np.exp(sc)
    attn = e / e.sum(axis=-1, keepdims=True)
    ctx = np.einsum('bhqk,bhkd->bhqd', attn, v)
    ctx = ctx.transpose(0, 2, 1, 3).reshape(B, S, D)
    return (np.einsum('bsd,ed->bse', ctx, Wo_w) + Wo_b).astype(np.float32)


def kernel(query, key, value, mask, rel_pos_bias, Wq, Wk, Wv, Wo_w, Wo_b):
    query = np.asarray(query, dtype=np.float32)
    key = np.asarray(key, dtype=np.float32)
    value = np.asarray(value, dtype=np.float32)
    mask = np.asarray(mask)
    rel_pos_bias = np.asarray(rel_pos_bias, dtype=np.float32)
    Wq = np.asarray(Wq, dtype=np.float32)
    Wk = np.asarray(Wk, dtype=np.float32)
    Wv = np.asarray(Wv, dtype=np.float32)
    Wo_w = np.asarray(Wo_w, dtype=np.float32)
    Wo_b = np.asarray(Wo_b, dtype=np.float32)

    if not np.array_equal(mask.reshape(S, S),
                          np.tril(np.ones((S, S), dtype=bool))):
        return _cpu_fallback(query, key, value, mask, rel_pos_bias,
                             Wq, Wk, Wv, Wo_w, Wo_b)

    out, _ = _run(query, key, value, rel_pos_bias, Wq, Wk, Wv, Wo_w, Wo_b)
    return out

